# revision 10
# baseline (speedup 1.0000x reference)
"""Bit2Num dequantization kernel for Trainium2 (8 NeuronCores, SPMD).

Reference op: x [1024, 65536] of {0.0, 1.0} f32, B=4.
  bits = x.reshape(1024, 16384, 4)
  out[b, n] = (8*bits[b,n,0] + 4*bits[b,n,1] + 2*bits[b,n,2] + bits[b,n,3] + 0.5) / 16

Sharding: pure data-parallel over batch — 128 rows per core (= 128 SBUF
partitions). Per core: 32 MB in + 8 MB out = 41.94 MB of HBM traffic.

Roofline (measured via semaphore-crossing cadence, not the duration-
diluted per-descriptor trace entries): the sustained per-NC DMA rate is
~405 GB/s of HBM-side bytes — two NCs share one ~819 GB/s HBM3 stack —
so the streaming window floor is ~104 us. On top of that the measured
exec window (gauge counts [first MEMSET .. last teardown op]) carries
~2 us of pre-stream framework ops and ~8.6 us of fixed NEFF/BSP teardown
(per-engine event-semaphore clear chains + final rendezvous) that kernel
structure cannot change. Best consistent exec ≈ 114-116 us = the floor.

Schedule: single SWDGE (gpsimd) load queue, f32->bf16 cast in-flight
(halves SBUF-side write traffic; keeps the 435 GB/s SBUF fabric
uncontended). Per chunk: 3 fused scalar_tensor_tensor ops on DVE
(u=2a+b, v=2c+d, w=4u+v over the 4 strided bit slices), final affine
(w/16 + 1/32) on ACT, store on the ACT HWDGE ring (separate queue, so
stores never stall loads).

Segment list is a ladder at BOTH ends:
  - Head 512/1024/2048 cols: the first small segments complete ~9-11 us
    instead of ~15.7 (a big first segment's completion is delayed by the
    SDMA packet round-robin over everything queued behind it), so the
    store stream starts ~11 us instead of ~22 — less store backlog at
    stream end (measured ~1.3 us faster than the flat layout).
  - Tail 2048/1024/1024/512: keeps the post-last-load compute+store
    cascade short (per-chunk latency is sem-hop/receipt dominated, so a
    few medium segments beat many tiny ones).

Hard-won constraints (measured, do not "improve"):
  - Do NOT put early loads on a second HWDGE queue: the two-queue packet
    round-robin settles the WHOLE stream into a ~352 GB/s limit cycle
    (131 us vs 115 us).
  - Do NOT shrink xin below ~8 bufs: load issue is gated on DVE progress
    (buffer recycle) and shallow pools starve the SDMA queue at the tail.
  - Keep total SBUF tiles <= ~190 KiB/partition: packing toward the
    224 KiB physical top (DMA scratch + reserves live there) corrupts
    tiles (NaN output).
"""

import numpy as np

import concourse.bacc as bacc
import concourse.bass as bass
import concourse.mybir as mybir
from concourse.bass_utils import run_bass_kernel_spmd
from concourse.tile import TileContext

N_CORES = 8
BATCH = 1024
COLS = 65536
B_BITS = 4
ROWS = BATCH // N_CORES          # 128 rows per core == SBUF partition count
OUT_COLS = COLS // B_BITS        # 16384

F32 = mybir.dt.float32
BF16 = mybir.dt.bfloat16
MULT = mybir.AluOpType.mult
ADD = mybir.AluOpType.add

# (in_cols, [chunk_out_cols...]) per segment; all SWDGE bf16 loads.
SEGMENTS = (
    [(512, [128]), (1024, [256]), (2048, [512])]
    + [(4096, [1024])] * 14
    + [(2048, [512]), (1024, [256]), (1024, [256]), (512, [128])]
)
assert sum(s[0] for s in SEGMENTS) == COLS


def _build_nc() -> bass.Bass:
    # Bacc (not plain Bass): its compile() pipeline runs
    # generate_event_semaphores, which splits multi-wait sync conditions —
    # TRN2 DMA instructions accept at most one wait.
    nc = bacc.Bacc(None, target_bir_lowering=False)
    x = nc.dram_tensor("x", [ROWS, COLS], F32, kind="ExternalInput")
    out = nc.dram_tensor("out", [ROWS, OUT_COLS], F32, kind="ExternalOutput")

    with TileContext(nc) as tc:
        with (
            tc.tile_pool(name="xin", bufs=8) as xpool,
            tc.tile_pool(name="work", bufs=4) as wpool,
            tc.tile_pool(name="wacc", bufs=4) as wxpool,
            tc.tile_pool(name="oout", bufs=4) as opool,
        ):
            col = 0
            g_off = 0
            for seg_c, chunk_gs in SEGMENTS:
                xt = xpool.tile([ROWS, seg_c], BF16, tag="xt")
                nc.gpsimd.dma_start(
                    out=xt[:, :], in_=x[:, col:col + seg_c]
                )
                col += seg_c
                c_off = 0
                for chunk_g in chunk_gs:
                    chunk_c = chunk_g * B_BITS
                    xv = xt[:, c_off:c_off + chunk_c].rearrange(
                        "p (g k) -> p g k", k=B_BITS
                    )
                    c_off += chunk_c
                    a = xv[:, :, 0]
                    b = xv[:, :, 1]
                    c = xv[:, :, 2]
                    d = xv[:, :, 3]

                    # intermediates stay bf16 (all values <= 15, exact);
                    # ACT casts back to f32 on the final affine.
                    u = wpool.tile([ROWS, chunk_g], BF16, tag="u")
                    v = wpool.tile([ROWS, chunk_g], BF16, tag="v")
                    w = wxpool.tile([ROWS, chunk_g], BF16, tag="w")
                    ot = opool.tile([ROWS, chunk_g], F32, tag="ot")

                    # u = 2a + b ; v = 2c + d ; w = 4u + v = 8a+4b+2c+d
                    nc.vector.scalar_tensor_tensor(
                        out=u[:, :], in0=a, scalar=2.0, in1=b,
                        op0=MULT, op1=ADD,
                    )
                    nc.vector.scalar_tensor_tensor(
                        out=v[:, :], in0=c, scalar=2.0, in1=d,
                        op0=MULT, op1=ADD,
                    )
                    nc.vector.scalar_tensor_tensor(
                        out=w[:, :], in0=u[:, :], scalar=4.0, in1=v[:, :],
                        op0=MULT, op1=ADD,
                    )
                    # ot = (w + 0.5) / 16 = w/16 + 1/32
                    nc.scalar.activation(
                        out=ot[:, :], in_=w[:, :],
                        func=mybir.ActivationFunctionType.Copy,
                        bias=1.0 / 32.0, scale=1.0 / 16.0,
                    )
                    # out-DMAs on the ACT HWDGE ring (qActDynamicHW) so a
                    # store waiting on compute never blocks the in-stream.
                    nc.scalar.dma_start(
                        out=out[:, g_off:g_off + chunk_g], in_=ot[:, :]
                    )
                    g_off += chunk_g
    # Bacc.finalize runs the compile pipeline (register allocation +
    # generate_event_semaphores); the pjrt exec path serializes nc.m as-is.
    nc.finalize()
    return nc


_NC = None


def _get_nc() -> bass.Bass:
    global _NC
    if _NC is None:
        _NC = _build_nc()
    return _NC


def kernel(x: np.ndarray, B=4) -> np.ndarray:
    assert int(B) == B_BITS, f"kernel hardcodes B={B_BITS}, got {B}"
    x = np.ascontiguousarray(x, dtype=np.float32)
    assert x.shape == (BATCH, COLS), x.shape
    nc = _get_nc()
    in_maps = [{"x": x[i * ROWS:(i + 1) * ROWS]} for i in range(N_CORES)]
    res = run_bass_kernel_spmd(nc, in_maps, list(range(N_CORES)))
    return np.concatenate(
        [res.results[i]["out"] for i in range(N_CORES)], axis=0
    )
